# revision 38
# baseline (speedup 1.0000x reference)
"""
Trainium2 distributed kernel for causal multi-head attention
(nn_Attention: B=4, S=2048, D=768, H=4, DH=192).

Sharding: 16 (batch, head) units across 8 cores = 1 batch x 2 heads per
core.  Every core runs an identical graph (SPMD) on its own shard; the
host sums core pairs (the unshard for output-partial sharding).  No
on-device collectives, perfectly balanced causal work.

Device algorithm (bf16 matmuls, f32 PSUM accumulation):
  QT/KT stored transposed [head-dim planes, seq]; the two heads' upper
  64 head-dims share one 128-partition plane (host permutes weight
  columns to match), so every projection matmul contracts a full 128
  partitions and the two 64-row score matmuls run in disjoint PE row
  groups (concurrent).  V is stored naturally [seq, dh] with per-head
  ones columns so the AV matmul also emits softmax denominators.
  Scores are computed transposed, S.T[k, q] = KT.T @ QT, so exp'd
  attention tiles feed AV directly as the moving operand — no
  transposes anywhere.  Softmax skips max-subtraction (logits are O(1)
  by construction); causality is applied post-exp as a multiplicative
  0/1 bf16 mask on the diagonal blocks only (fast DVE mode, off the
  PSUM->exp chain); fully-masked blocks are never computed, and the
  diagonal blocks' score/exp/mask/AV work is trimmed to the live
  query suffix [128*d:].

  Normalization is division-free on the hot path: head h's preout is
  scaled by the OTHER head's raw denominator row (broadcast via a
  ones-matmul), and the head-summed out-proj rows are multiplied by
  1/(d0*d1) — computed once per q-block as a [q-part, 4] transposed
  tile (contraction-1 matmuls), so the DVE reciprocal touches only 4
  elems/lane.  The scale rides the out-proj PSUM->SBUF move as a
  per-partition tensor_scalar (DVE) / activation-scale (ACT) operand.

  Scheduling: V-projection runs after the first score block so ScalarE
  digests the 16-exp backlog under PE cover; out-proj psums borrow the
  idle AV psum banks where legal; proj copies alternate ACT/DVE; dummy
  matmuls during the DMA lead-in pre-open the HAM clock gate; the
  final block drains per-half with alternating copy engines.  Output
  is bf16 (host sums core pairs in fp32).
"""

import math
import os
import sys

import numpy as np

for _p in ("/opt/trn_rl_repo",):
    if _p not in sys.path and os.path.isdir(_p):
        sys.path.insert(0, _p)

import ml_dtypes  # noqa: E402

B, S, D, H = 4, 2048, 768, 4
DH = D // H  # 192
HPC = 2  # heads per core
HD = HPC * DH  # 384 head dims per core
P = 128
KD = D // P  # 6 contraction chunks over D
QB = 512  # query block (matmul moving dim)
NQ = S // QB  # 4
KB = 128  # key block (psum partition dim)
NK = S // KB  # 16
MS = S // P  # 16 seq chunks
SCALE = 1.0 / math.sqrt(DH)
MASK_NEG = -1e9

# host-side column permutation for Wq/Wk (and row perm for Wo):
# planes = [h0 dh0:128 | h1 dh0:128 | h0 dh128:192, h1 dh128:192]
PQ = np.r_[0:128, 192:320, 128:192, 320:384]
# for Wv: [h0 dh0:192 | h1 dh128:192 | h1 dh0:128] so that the SBUF V
# tile [.. h0dh(192), ones0, ones1, h1dh128:192(64), h1dh0:128(128)]
# fills with two contiguous copies
PV = np.r_[0:192, 320:384, 192:320]

_CACHED = {}


def build_nc(reps=1, use_pool=False, sc2=False, actcopy=True):
    import concourse.mybir as mybir
    from concourse import bacc
    from concourse import tile

    fp32 = mybir.dt.float32
    bf16 = mybir.dt.bfloat16
    Exp = mybir.ActivationFunctionType.Exp

    nc = bacc.Bacc(None, target_bir_lowering=False)

    xT = nc.declare_dram_parameter("xT", [D, S], bf16, isOutput=False)
    wqT = nc.declare_dram_parameter("wqT", [D, HD], bf16, isOutput=False)
    wkT = nc.declare_dram_parameter("wkT", [D, HD], bf16, isOutput=False)
    wvT = nc.declare_dram_parameter("wvT", [D, HD], bf16, isOutput=False)
    woS = nc.declare_dram_parameter("woS", [HD, D], bf16, isOutput=False)
    out = nc.declare_dram_parameter("out", [S, D], bf16, isOutput=True)

    # V sbuf free-layout offsets
    V_H0C0 = slice(0, 128)
    V_H0C1 = slice(128, 193)  # h0 dh128:192 + ones0 @192 -> denom row 64
    V_H1C1 = slice(193, 258)  # h1 dh128:192 + ones1 @257 -> denom row 64
    V_H1C0 = slice(258, 386)
    VW = 386

    with tile.TileContext(nc) as tc:
        with (
            tc.tile_pool(name="const", bufs=1) as const,
            tc.tile_pool(name="atp", bufs=2) as atp,
            tc.tile_pool(name="ost", bufs=3) as ostp,
            tc.tile_pool(name="rcp", bufs=2) as rcp,
            tc.tile_pool(name="scps", bufs=2, space="PSUM") as scps,
            tc.tile_pool(name="avps", bufs=1, space="PSUM") as avps,
        ):
            # ---- persistent SBUF tensors ----
            xT_sb = const.tile([P, KD, S], bf16, tag="xT_sb")
            rinv_sb = [
                const.tile([P, 4], fp32, tag=f"rinv{j}", name=f"rinv{j}")
                for j in range(NQ)
            ]
            wq_sb = const.tile([P, KD, HD], bf16, tag="wq_sb")
            wk_sb = const.tile([P, KD, HD], bf16, tag="wk_sb")
            wv_sb = const.tile([P, KD, HD], bf16, tag="wv_sb")
            wo_sb = const.tile([P, 3, D], bf16, tag="wo_sb")
            qt_sb = const.tile([P, 3, S], bf16, tag="qt_sb")
            kt_sb = const.tile([P, 3, S], bf16, tag="kt_sb")
            v_sb = const.tile([P, NK, VW], bf16, tag="v_sb")
            pt_sb = const.tile([P, 3, S], bf16, tag="pt_sb")
            ones1 = const.tile([1, P], bf16, tag="ones1")
            warm = const.tile([1, 1], fp32, tag="warm")
            masks = const.tile([P, 4, 2 * QB], bf16, tag="masks")

            # ---- input DMAs ----
            # one big DMA per weight (issue overhead ~0.7us each), xT
            # planes spread over three queues so plane k lands early
            # wq per-plane so the first wave's k-th matmuls gate on just
            # plane k; wk whole (one issue, lands before its first use);
            # xT planes spread so plane k lands just ahead of consumption
            for k in range(KD):
                nc.scalar.dma_start(
                    wq_sb[:, k, :], wqT[k * P : (k + 1) * P, :]
                )
            nc.gpsimd.dma_start(
                wk_sb[:], wkT.rearrange("(ko ki) j -> ki ko j", ki=P)
            )
            for k, eng in enumerate(
                (nc.sync, nc.sync, nc.gpsimd, nc.sync, nc.scalar, nc.sync)
            ):
                eng.dma_start(xT_sb[:, k, :], xT[k * P : (k + 1) * P, :])
            nc.scalar.dma_start(
                wv_sb[:], wvT.rearrange("(ko ki) j -> ki ko j", ki=P)
            )
            nc.gpsimd.dma_start(
                wo_sb[:], woS.rearrange("(ko ki) j -> ki ko j", ki=P)
            )

            nc.vector.memset(ones1[:], 1.0)
            # prefetch the exp table while the PE does projections
            nc.scalar.activation(warm[:], ones1[0:1, 0:1], Exp)
            # PE pre-warm: dummy matmuls during the input-DMA lead-in so
            # the HAM clock gate opens (1.2->2.4GHz) before real work
            dmy = const.tile([P, QB], bf16, tag="dmy")
            nc.vector.memset(dmy[:], 0.0)
            dmyps = scps.tile([P, 2 * QB], fp32, tag="sc", name="dmyps")
            for i in range(8):
                nc.tensor.matmul(
                    dmyps[:, 0:QB], lhsT=dmy[:, 0:P], rhs=dmy[:],
                    start=(i == 0), stop=(i == 7),
                )
            # ones columns of V are static: set them once
            nc.vector.memset(v_sb[:, :, 192:193], 1.0)
            nc.vector.memset(v_sb[:, :, 257:258], 1.0)

            # multiplicative causal masks (0/1 bf16) for the 4 diagonal
            # sub-blocks, double width to cover both heads' fused at tile:
            # keep 1 iff q_local >= 128*d + k_local, else 0
            for d in range(4):
                nc.vector.memset(masks[:, d, :], 1.0)
                nc.gpsimd.affine_select(
                    out=masks[:, d, :],
                    in_=masks[:, d, :],
                    compare_op=mybir.AluOpType.is_ge,
                    fill=0.0,
                    base=-128 * d,
                    pattern=[[0, 2], [1, QB]],
                    channel_multiplier=-1,
                )

            # ---- Q/K projections (transposed outputs, 3 full planes) ----
            # alternate psum->sbuf proj copies between ACT and DVE: both
            # engines are otherwise idle during projections, and a single
            # engine's copy chain stalls the bufs=1 psum tag reuse
            _alt = [0]

            def alt_copy(dst, src):
                _alt[0] ^= 1
                (nc.scalar.copy if _alt[0] else nc.vector.tensor_copy)(
                    dst, src
                )

            def proj_wave(w_sb, o_sb, c):
                # k-outer over 4 simultaneous psum groups: consumes xT
                # planes as they arrive from DRAM (cuts the DMA lead-in)
                pss = [
                    avps.tile([P, QB], fp32, tag=t, name=f"wave_{t}")
                    for t in ("avA", "avB", "avC", "avD")
                ]
                for k in range(KD):
                    for nt in range(NQ):
                        nc.tensor.matmul(
                            pss[nt],
                            lhsT=w_sb[:, k, c * P : (c + 1) * P],
                            rhs=xT_sb[:, k, nt * QB : (nt + 1) * QB],
                            start=(k == 0),
                            stop=(k == KD - 1),
                        )
                for nt in range(NQ):
                    alt_copy(o_sb[:, c, nt * QB : (nt + 1) * QB], pss[nt])

            def wide_wave():
                # per xT plane: Q-c0's 4 groups (av tags) + K-c0's first 2
                # groups (sc slots) -> ~6 matmuls per plane arrival
                pssQ = [
                    avps.tile([P, QB], fp32, tag=t, name=f"wwq_{t}")
                    for t in ("avA", "avB", "avC", "avD")
                ]
                pssK = [
                    scps.tile([P, QB], fp32, tag="sc", name=f"wwk_{i}")
                    for i in range(2)
                ]
                for k in range(KD):
                    for nt in range(NQ):
                        nc.tensor.matmul(
                            pssQ[nt],
                            lhsT=wq_sb[:, k, 0:P],
                            rhs=xT_sb[:, k, nt * QB : (nt + 1) * QB],
                            start=(k == 0), stop=(k == KD - 1),
                        )
                    for nt in range(2):
                        nc.tensor.matmul(
                            pssK[nt],
                            lhsT=wk_sb[:, k, 0:P],
                            rhs=xT_sb[:, k, nt * QB : (nt + 1) * QB],
                            start=(k == 0), stop=(k == KD - 1),
                        )
                for nt in range(NQ):
                    alt_copy(qt_sb[:, 0, nt * QB : (nt + 1) * QB], pssQ[nt])
                for nt in range(2):
                    alt_copy(kt_sb[:, 0, nt * QB : (nt + 1) * QB], pssK[nt])
                # K-c0's remaining 2 groups (planes all resident by now)
                for nt in (2, 3):
                    ps = avps.tile(
                        [P, QB], fp32, tag="av" + "ABCD"[nt], name=f"kc0{nt}"
                    )
                    for k in range(KD):
                        nc.tensor.matmul(
                            ps,
                            lhsT=wk_sb[:, k, 0:P],
                            rhs=xT_sb[:, k, nt * QB : (nt + 1) * QB],
                            start=(k == 0), stop=(k == KD - 1),
                        )
                    alt_copy(kt_sb[:, 0, nt * QB : (nt + 1) * QB], ps)

            def projections(first=False):
                if first:
                    wide_wave()
                for w_sb, o_sb in ((wq_sb, qt_sb), (wk_sb, kt_sb)):
                    for c in range(1 if first else 0, 3):
                        for nt in range(NQ):
                            ps = avps.tile(
                                [P, QB], fp32,
                                tag="av" + "ABCD"[nt], name=f"pj{c}{nt}",
                            )
                            for k in range(KD):
                                nc.tensor.matmul(
                                    ps,
                                    lhsT=w_sb[:, k, c * P : (c + 1) * P],
                                    rhs=xT_sb[:, k, nt * QB : (nt + 1) * QB],
                                    start=(k == 0),
                                    stop=(k == KD - 1),
                                )
                            alt_copy(
                                o_sb[:, c, nt * QB : (nt + 1) * QB], ps
                            )

            def v_proj():
                # ---- V projection (natural layout) + ones columns ----
                # runs AFTER the first score block: its 15us of PE work
                # covers the ScalarE exp backlog for qj=3
                for m in range(MS):
                    ps = avps.tile(
                        [P, QB], fp32, tag="av" + "ABCD"[m % 4], name=f"pv{m}"
                    )
                    for k in range(KD):
                        nc.tensor.matmul(
                            ps[:, 0:HD],
                            lhsT=xT_sb[:, k, m * P : (m + 1) * P],
                            rhs=wv_sb[:, k, :],
                            start=(k == 0),
                            stop=(k == KD - 1),
                        )
                    (nc.scalar.copy if actcopy else nc.vector.tensor_copy)(
                        v_sb[:, m, 0:192], ps[:, 0:192]
                    )
                    nc.vector.tensor_copy(v_sb[:, m, 193:257], ps[:, 192:256])
                    nc.vector.tensor_copy(v_sb[:, m, 258:386], ps[:, 256:384])

            # ---- attention per q-block; out-proj deferred one block ----
            def out_proj(qj, mis=(0, 1, 2, 3), on_act=False, tail=False,
                         on_avps=False):
                Copy = mybir.ActivationFunctionType.Copy
                for mi in mis:
                    m = qj * 4 + mi
                    ost = ostp.tile([P, D], bf16, tag="ost")
                    rinv1 = rinv_sb[qj][:, mi : mi + 1]
                    for n in range(2):
                        # when the AV psum tags are idle, use them for the
                        # out-proj psum: the "sc" pool is recycled through
                        # the exp backlog and stalls the allocation
                        if on_avps:
                            ps = avps.tile(
                                [P, QB], fp32,
                                tag=("avB" if n == 0 else "avD"),
                                name=f"op{qj}{mi}{n}",
                            )
                        else:
                            ps = scps.tile(
                                [P, QB], fp32, tag="sc", name=f"op{mi}{n}"
                            )
                        for c in range(3):
                            nc.tensor.matmul(
                                ps[:, 0:384],
                                lhsT=pt_sb[:, c, m * P : (m + 1) * P],
                                rhs=wo_sb[:, c, n * 384 : (n + 1) * 384],
                                start=(c == 0),
                                stop=(c == 2),
                            )
                        # psum->sbuf move fused with the 1/(d0*d1) row scale
                        h = 2 * mi + n
                        if (h % 2 == 1) if tail else on_act:
                            nc.scalar.activation(
                                ost[:, n * 384 : (n + 1) * 384],
                                ps[:, 0:384], Copy, scale=rinv1,
                            )
                        else:
                            nc.vector.tensor_scalar_mul(
                                ost[:, n * 384 : (n + 1) * 384],
                                ps[:, 0:384], rinv1,
                            )
                        if tail:
                            # final block: drain each half immediately so
                            # the copy chain and DMAs overlap
                            nsl = slice(n * 384, (n + 1) * 384)
                            [nc.sync, nc.gpsimd, nc.scalar][h % 3].dma_start(
                                out[m * P : (m + 1) * P, nsl], ost[:, nsl]
                            )
                    if not tail:
                        # spread output DMAs over queues: the final drain
                        # barrier waits on them
                        [nc.sync, nc.scalar, nc.gpsimd][m % 3].dma_start(
                            out[m * P : (m + 1) * P, :], ost[:]
                        )

            def scores_part(qj):
                nk = 4 * qj + 4  # live key blocks (causal)
                # fused at tile: both heads side by side [.., h0 512 | h1 512]
                at2 = atp.tile(
                    [P, NK, 2 * QB], bf16, tag="at2", name=f"at2_{qj}"
                )
                for ki in range(nk):
                    ksl = slice(ki * KB, (ki + 1) * KB)
                    d = ki - 4 * qj
                    # diagonal blocks: queries below 128*d are fully masked
                    # -- trim the moving dim to the live suffix [t:QB]
                    t = max(0, d) * KB
                    qsl = slice(qj * QB + t, (qj + 1) * QB)
                    # one 2-bank psum tile holds both heads' score block
                    ps = scps.tile([P, 2 * QB], fp32, tag="sc")
                    ps0 = ps[:, t:QB]
                    ps1 = ps[:, QB + t : 2 * QB]
                    # full-plane matmuls (128 contraction rows); their
                    # LDWEIGHTS hide in the background weight buffer
                    nc.tensor.matmul(
                        ps0, lhsT=kt_sb[:, 0, ksl], rhs=qt_sb[:, 0, qsl],
                        start=True, stop=False,
                    )
                    nc.tensor.matmul(
                        ps1, lhsT=kt_sb[:, 1, ksl], rhs=qt_sb[:, 1, qsl],
                        start=True, stop=False,
                    )
                    # 64-row tails in disjoint row groups (concurrent)
                    nc.tensor.matmul(
                        ps0, lhsT=kt_sb[0:64, 2, ksl], rhs=qt_sb[0:64, 2, qsl],
                        start=False, stop=True,
                    )
                    nc.tensor.matmul(
                        ps1,
                        lhsT=kt_sb[64:128, 2, ksl],
                        rhs=qt_sb[64:128, 2, qsl],
                        start=False, stop=True,
                    )
                    # one exp for both heads: amortizes the ACT ramp.  For
                    # trimmed diagonal blocks a 2-segment AP covers just the
                    # live columns of both heads.
                    if t == 0:
                        nc.scalar.activation(
                            at2[:, ki, :], ps, Exp, scale=SCALE
                        )
                        a2m = at2[:, ki, :]
                        msk = masks[:, d, :] if d >= 0 else None
                    else:
                        psv = ps.rearrange("p (h q) -> p h q", h=2)[:, :, t:QB]
                        a2m = at2[:, ki, :].rearrange(
                            "p (h q) -> p h q", h=2
                        )[:, :, t:QB]
                        msk = masks[:, d, :].rearrange(
                            "p (h q) -> p h q", h=2
                        )[:, :, t:QB]
                        nc.scalar.activation(a2m, psv, Exp, scale=SCALE)
                    if d >= 0:
                        # multiplicative causal zeroing post-exp: bf16 SBUF
                        # DVE fast mode, off the PSUM->exp chain
                        nc.vector.tensor_mul(a2m, a2m, msk)
                return at2

            def av_part(qj, at2, mid=None):
                qsl = slice(qj * QB, (qj + 1) * QB)
                nk = 4 * qj + 4
                at0 = at2[:, :, 0:QB]
                at1 = at2[:, :, QB : 2 * QB]

                def av(h, at, c0sl, c1sl, tagA, tagC):
                    psc0 = avps.tile(
                        [P, QB], fp32, tag=tagA, name=f"av0_{qj}{h}"
                    )
                    psc1 = avps.tile(
                        [P, QB], fp32, tag=tagC, name=f"av1_{qj}{h}"
                    )
                    for ki in range(nk):
                        # diagonal blocks contribute zeros below q=128*d:
                        # skip those columns (they were masked out anyway)
                        t = max(0, ki - 4 * qj) * KB
                        nc.tensor.matmul(
                            psc0[:, t:QB], lhsT=v_sb[:, ki, c0sl],
                            rhs=at[:, ki, t:QB],
                            start=(ki == 0), stop=(ki == nk - 1),
                        )
                        nc.tensor.matmul(
                            psc1[0:65, t:QB], lhsT=v_sb[:, ki, c1sl],
                            rhs=at[:, ki, t:QB],
                            start=(ki == 0), stop=(ki == nk - 1),
                        )
                    return psc0, psc1

                # Division-free per-head normalization: scale head h's
                # preout by the OTHER head's raw denominator, then divide
                # the head-summed out-proj rows by d0*d1 (a per-partition
                # scalar there).  The only reciprocal runs on a PE-
                # transposed [128,4] tile -- 4 elems/lane, ~160ns.
                def norm(h, psc0, psc1, rcb_ps, d_other):
                    rcb = rcp.tile([P, QB], fp32, tag="rcb")
                    nc.tensor.matmul(
                        rcb_ps, lhsT=ones1[:], rhs=d_other[:],
                        start=True, stop=True,
                    )
                    # DVE, not ACT: ACT is still draining exps here
                    nc.vector.tensor_copy(rcb[:], rcb_ps)
                    nc.vector.tensor_mul(pt_sb[:, h, qsl], psc0, rcb[:])
                    # upper 64 head dims land in plane 2: h0 -> partitions
                    # 0:64, h1 -> partitions 64:128 (partition-shifted write)
                    if h == 0:
                        nc.vector.tensor_mul(
                            pt_sb[0:64, 2, qsl], psc1[0:64], rcb[0:64]
                        )
                    else:
                        nc.vector.tensor_mul(
                            pt_sb[64:128, 2, qsl], psc1[0:64], rcb[64:128]
                        )

                psA, psC = av(0, at0, V_H0C0, V_H0C1, "avA", "avC")
                psB, psD = av(1, at1, V_H1C0, V_H1C1, "avB", "avD")
                d0r = rcp.tile([1, QB], bf16, tag="d0r")
                d1r = rcp.tile([1, QB], bf16, tag="d1r")
                nc.vector.tensor_copy(d0r[:], psC[64:65, :])
                nc.vector.tensor_copy(d1r[:], psD[64:65, :])
                if mid is not None:
                    mid()  # independent PE work to cover the norm chain
                # both heads' broadcasts share ONE 2-bank slot so the other
                # score slot is free for the next block's first matmuls
                rcb2 = scps.tile(
                    [P, 2 * QB], fp32, tag="sc", name=f"rcb2_{qj}"
                )
                # head 1 first: its psums (avB/avD) are the ones the next
                # out_proj block reuses, so release them earliest
                norm(1, psB, psD, rcb2[:, QB : 2 * QB], d0r)
                norm(0, psA, psC, rcb2[:, 0:QB], d1r)
                # 1/(d0*d1) in transposed [q-part, chunk] layout for the
                # out-proj per-partition scale; bf16 lhsT keeps the
                # contraction-1 transpose matmuls at full rate
                d01 = rcp.tile([1, QB], bf16, tag="d01")
                nc.vector.tensor_mul(d01[:], d0r[:], d1r[:])
                dT = avps.tile([P, QB], fp32, tag="avC", name=f"dT_{qj}")
                for c in range(4):
                    nc.tensor.matmul(
                        dT[:, c : c + 1],
                        lhsT=d01[0:1, c * P : (c + 1) * P],
                        rhs=ones1[0:1, 0:1],
                        start=True, stop=True,
                    )
                nc.vector.reciprocal(rinv_sb[qj][:], dT[:, 0:4])

            # big q-blocks first; every AV waits one-block-deferred so the
            # next block's scores cover its exp tail, and out-proj halves
            # cover the norm chains
            for _rep in range(reps):
                projections(first=(_rep == 0))
                a3 = scores_part(3)
                v_proj()
                a2 = scores_part(2)
                av_part(3, a3)
                a1 = scores_part(1)
                out_proj(3, (0, 1), on_avps=True)
                av_part(2, a2)
                a0 = scores_part(0)
                out_proj(3, (2, 3), on_avps=True)
                out_proj(2, (0, 1), on_avps=True)
                av_part(1, a1)
                out_proj(2, (2, 3))
                av_part(
                    0, a0,
                    mid=lambda: out_proj(1, (0, 1), on_act=True),
                )
                out_proj(1, (2, 3), on_act=True)
                out_proj(0, tail=True, on_avps=True)

    nc.compile()
    return nc


def _shard_inputs(x, Wq, Wk, Wv, Wo):
    bf = ml_dtypes.bfloat16
    in_maps = []
    for core in range(8):
        b, hp = core // 2, core % 2
        cols = slice(hp * HD, (hp + 1) * HD)
        in_maps.append(
            {
                "xT": np.ascontiguousarray(x[b].T).astype(bf),
                "wqT": np.ascontiguousarray(Wq[cols, :].T[:, PQ]).astype(bf),
                "wkT": np.ascontiguousarray(Wk[cols, :].T[:, PQ]).astype(bf),
                "wvT": np.ascontiguousarray(Wv[cols, :].T[:, PV]).astype(bf),
                "woS": np.ascontiguousarray(Wo[:, cols].T[PQ, :]).astype(bf),
            }
        )
    return in_maps


def _run(inputs, trace=False, **kw):
    from concourse.bass_utils import run_bass_kernel_spmd

    if "nc" not in _CACHED:
        _CACHED["nc"] = build_nc()
    nc = _CACHED["nc"]
    in_maps = _shard_inputs(
        np.asarray(inputs["x"], np.float32),
        np.asarray(inputs["Wq"], np.float32),
        np.asarray(inputs["Wk"], np.float32),
        np.asarray(inputs["Wv"], np.float32),
        np.asarray(inputs["Wo"], np.float32),
    )
    res = run_bass_kernel_spmd(
        nc, in_maps, core_ids=list(range(8)), trace=trace, **kw
    )
    parts = [np.asarray(r["out"], np.float32) for r in res.results]
    full = np.empty((B, S, D), np.float32)
    for b in range(B):
        full[b] = parts[2 * b] + parts[2 * b + 1]
    return full, res


def kernel(**inputs) -> np.ndarray:
    full, _ = _run(inputs, trace=False)
    return full



# revision 40
# speedup vs baseline: 1.1793x; 1.1793x over previous
"""
Trainium2 distributed kernel for causal multi-head attention
(nn_Attention: B=4, S=2048, D=768, H=4, DH=192).

Sharding: 16 (batch, head) units across 8 cores = 1 batch x 2 heads per
core.  Every core runs an identical graph (SPMD) on its own shard; the
host sums core pairs (the unshard for output-partial sharding).  No
on-device collectives, perfectly balanced causal work.

Device algorithm (bf16 matmuls, f32 PSUM accumulation):
  QT/KT stored transposed [head-dim planes, seq]; the two heads' upper
  64 head-dims share one 128-partition plane (host permutes weight
  columns to match), so every projection matmul contracts a full 128
  partitions and the two 64-row score matmuls run in disjoint PE row
  groups (concurrent).  V is stored naturally [seq, dh] with per-head
  ones columns so the AV matmul also emits softmax denominators.
  Scores are computed transposed, S.T[k, q] = KT.T @ QT, so exp'd
  attention tiles feed AV directly as the moving operand — no
  transposes anywhere.  Softmax skips max-subtraction (logits are O(1)
  by construction); causality is applied post-exp as a multiplicative
  0/1 bf16 mask on the diagonal blocks only (fast DVE mode, off the
  PSUM->exp chain); fully-masked blocks are never computed, and the
  diagonal blocks' score/exp/mask/AV work is trimmed to the live
  query suffix [128*d:].

  Normalization is division-free on the hot path: head h's preout is
  scaled by the OTHER head's raw denominator row (broadcast via a
  ones-matmul), and the head-summed out-proj rows are multiplied by
  1/(d0*d1) — computed once per q-block as a [q-part, 4] transposed
  tile (contraction-1 matmuls), so the DVE reciprocal touches only 4
  elems/lane.  The scale rides the out-proj PSUM->SBUF move as a
  per-partition tensor_scalar (DVE) / activation-scale (ACT) operand.

  Scheduling: V-projection runs after the first score block so ScalarE
  digests the 16-exp backlog under PE cover; out-proj psums borrow the
  idle AV psum banks where legal; proj copies alternate ACT/DVE; dummy
  matmuls during the DMA lead-in pre-open the HAM clock gate; the
  final block drains per-half with alternating copy engines.  Output
  is bf16 (host sums core pairs in fp32).
"""

import math
import os
import sys

import numpy as np

for _p in ("/opt/trn_rl_repo",):
    if _p not in sys.path and os.path.isdir(_p):
        sys.path.insert(0, _p)

import ml_dtypes  # noqa: E402

B, S, D, H = 4, 2048, 768, 4
DH = D // H  # 192
HPC = 2  # heads per core
HD = HPC * DH  # 384 head dims per core
P = 128
KD = D // P  # 6 contraction chunks over D
QB = 512  # query block (matmul moving dim)
NQ = S // QB  # 4
KB = 128  # key block (psum partition dim)
NK = S // KB  # 16
MS = S // P  # 16 seq chunks
SCALE = 1.0 / math.sqrt(DH)
MASK_NEG = -1e9

# host-side column permutation for Wq/Wk (and row perm for Wo):
# planes = [h0 dh0:128 | h1 dh0:128 | h0 dh128:192, h1 dh128:192]
PQ = np.r_[0:128, 192:320, 128:192, 320:384]
# for Wv: [h0 dh0:192 | h1 dh128:192 | h1 dh0:128] so that the SBUF V
# tile [.. h0dh(192), ones0, ones1, h1dh128:192(64), h1dh0:128(128)]
# fills with two contiguous copies
PV = np.r_[0:192, 320:384, 192:320]

_CACHED = {}


def build_nc(reps=1, use_pool=False, sc2=False, actcopy=True):
    import concourse.mybir as mybir
    from concourse import bacc
    from concourse import tile

    fp32 = mybir.dt.float32
    bf16 = mybir.dt.bfloat16
    Exp = mybir.ActivationFunctionType.Exp

    nc = bacc.Bacc(None, target_bir_lowering=False)

    xT = nc.declare_dram_parameter("xT", [D, S], bf16, isOutput=False)
    wqT = nc.declare_dram_parameter("wqT", [D, HD], bf16, isOutput=False)
    wkT = nc.declare_dram_parameter("wkT", [D, HD], bf16, isOutput=False)
    wvT = nc.declare_dram_parameter("wvT", [D, HD], bf16, isOutput=False)
    woS = nc.declare_dram_parameter("woS", [HD, D], bf16, isOutput=False)
    out = nc.declare_dram_parameter("out", [S, D], bf16, isOutput=True)

    # V sbuf free-layout offsets
    V_H0C0 = slice(0, 128)
    V_H0C1 = slice(128, 193)  # h0 dh128:192 + ones0 @192 -> denom row 64
    V_H1C1 = slice(193, 258)  # h1 dh128:192 + ones1 @257 -> denom row 64
    V_H1C0 = slice(258, 386)
    VW = 386

    with tile.TileContext(nc) as tc:
        with (
            tc.tile_pool(name="const", bufs=1) as const,
            tc.tile_pool(name="atp", bufs=2) as atp,
            tc.tile_pool(name="ost", bufs=3) as ostp,
            tc.tile_pool(name="rcp", bufs=2) as rcp,
            tc.tile_pool(name="scps", bufs=2, space="PSUM") as scps,
            tc.tile_pool(name="avps", bufs=1, space="PSUM") as avps,
        ):
            # ---- persistent SBUF tensors ----
            xT_sb = const.tile([P, KD, S], bf16, tag="xT_sb")
            rinv_sb = [
                const.tile([P, 4], fp32, tag=f"rinv{j}", name=f"rinv{j}")
                for j in range(NQ)
            ]
            wq_sb = const.tile([P, KD, HD], bf16, tag="wq_sb")
            wk_sb = const.tile([P, KD, HD], bf16, tag="wk_sb")
            wv_sb = const.tile([P, KD, HD], bf16, tag="wv_sb")
            wo_sb = const.tile([P, 3, D], bf16, tag="wo_sb")
            qt_sb = const.tile([P, 3, S], bf16, tag="qt_sb")
            kt_sb = const.tile([P, 3, S], bf16, tag="kt_sb")
            v_sb = const.tile([P, NK, VW], bf16, tag="v_sb")
            pt_sb = const.tile([P, 3, S], bf16, tag="pt_sb")
            ones1 = const.tile([1, P], bf16, tag="ones1")
            warm = const.tile([1, 1], fp32, tag="warm")
            masks = const.tile([P, 4, 2 * QB], bf16, tag="masks")

            # ---- input DMAs ----
            # one big DMA per weight (issue overhead ~0.7us each), xT
            # planes spread over three queues so plane k lands early
            # wq per-plane so the first wave's k-th matmuls gate on just
            # plane k; wk whole (one issue, lands before its first use);
            # xT planes spread so plane k lands just ahead of consumption
            for k in range(KD):
                nc.scalar.dma_start(
                    wq_sb[:, k, :], wqT[k * P : (k + 1) * P, :]
                )
            nc.gpsimd.dma_start(
                wk_sb[:], wkT.rearrange("(ko ki) j -> ki ko j", ki=P)
            )
            for k, eng in enumerate(
                (nc.sync, nc.sync, nc.gpsimd, nc.sync, nc.scalar, nc.sync)
            ):
                eng.dma_start(xT_sb[:, k, :], xT[k * P : (k + 1) * P, :])
            nc.scalar.dma_start(
                wv_sb[:], wvT.rearrange("(ko ki) j -> ki ko j", ki=P)
            )
            nc.gpsimd.dma_start(
                wo_sb[:], woS.rearrange("(ko ki) j -> ki ko j", ki=P)
            )

            nc.vector.memset(ones1[:], 1.0)
            # prefetch the exp table while the PE does projections
            nc.scalar.activation(warm[:], ones1[0:1, 0:1], Exp)
            # PE pre-warm: dummy matmuls during the input-DMA lead-in so
            # the HAM clock gate opens (1.2->2.4GHz) before real work
            dmy = const.tile([P, QB], bf16, tag="dmy")
            nc.vector.memset(dmy[:], 0.0)
            dmyps = scps.tile([P, 2 * QB], fp32, tag="sc", name="dmyps")
            for i in range(8):
                nc.tensor.matmul(
                    dmyps[:, 0:QB], lhsT=dmy[:, 0:P], rhs=dmy[:],
                    start=(i == 0), stop=(i == 7),
                )
            # ones columns of V are static: set them once
            nc.vector.memset(v_sb[:, :, 192:193], 1.0)
            nc.vector.memset(v_sb[:, :, 257:258], 1.0)

            # multiplicative causal masks (0/1 bf16) for the 4 diagonal
            # sub-blocks, double width to cover both heads' fused at tile:
            # keep 1 iff q_local >= 128*d + k_local, else 0
            for d in range(4):
                nc.vector.memset(masks[:, d, :], 1.0)
                nc.gpsimd.affine_select(
                    out=masks[:, d, :],
                    in_=masks[:, d, :],
                    compare_op=mybir.AluOpType.is_ge,
                    fill=0.0,
                    base=-128 * d,
                    pattern=[[0, 2], [1, QB]],
                    channel_multiplier=-1,
                )

            # ---- Q/K projections (transposed outputs, 3 full planes) ----
            # alternate psum->sbuf proj copies between ACT and DVE: both
            # engines are otherwise idle during projections, and a single
            # engine's copy chain stalls the bufs=1 psum tag reuse
            _alt = [0]

            def alt_copy(dst, src):
                _alt[0] ^= 1
                (nc.scalar.copy if _alt[0] else nc.vector.tensor_copy)(
                    dst, src
                )

            def proj_wave(w_sb, o_sb, c):
                # k-outer over 4 simultaneous psum groups: consumes xT
                # planes as they arrive from DRAM (cuts the DMA lead-in)
                pss = [
                    avps.tile([P, QB], fp32, tag=t, name=f"wave_{t}")
                    for t in ("avA", "avB", "avC", "avD")
                ]
                for k in range(KD):
                    for nt in range(NQ):
                        nc.tensor.matmul(
                            pss[nt],
                            lhsT=w_sb[:, k, c * P : (c + 1) * P],
                            rhs=xT_sb[:, k, nt * QB : (nt + 1) * QB],
                            start=(k == 0),
                            stop=(k == KD - 1),
                        )
                for nt in range(NQ):
                    alt_copy(o_sb[:, c, nt * QB : (nt + 1) * QB], pss[nt])

            def wide_wave():
                # per xT plane: Q-c0's 4 groups (av tags) + K-c0's first 2
                # groups (sc slots) -> ~6 matmuls per plane arrival
                pssQ = [
                    avps.tile([P, QB], fp32, tag=t, name=f"wwq_{t}")
                    for t in ("avA", "avB", "avC", "avD")
                ]
                pssK = [
                    scps.tile([P, QB], fp32, tag="sc", name=f"wwk_{i}")
                    for i in range(2)
                ]
                for k in range(KD):
                    for nt in range(NQ):
                        nc.tensor.matmul(
                            pssQ[nt],
                            lhsT=wq_sb[:, k, 0:P],
                            rhs=xT_sb[:, k, nt * QB : (nt + 1) * QB],
                            start=(k == 0), stop=(k == KD - 1),
                        )
                    for nt in range(2):
                        nc.tensor.matmul(
                            pssK[nt],
                            lhsT=wk_sb[:, k, 0:P],
                            rhs=xT_sb[:, k, nt * QB : (nt + 1) * QB],
                            start=(k == 0), stop=(k == KD - 1),
                        )
                for nt in range(NQ):
                    alt_copy(qt_sb[:, 0, nt * QB : (nt + 1) * QB], pssQ[nt])
                for nt in range(2):
                    alt_copy(kt_sb[:, 0, nt * QB : (nt + 1) * QB], pssK[nt])
                # K-c0's remaining 2 groups (planes all resident by now)
                for nt in (2, 3):
                    ps = avps.tile(
                        [P, QB], fp32, tag="av" + "ABCD"[nt], name=f"kc0{nt}"
                    )
                    for k in range(KD):
                        nc.tensor.matmul(
                            ps,
                            lhsT=wk_sb[:, k, 0:P],
                            rhs=xT_sb[:, k, nt * QB : (nt + 1) * QB],
                            start=(k == 0), stop=(k == KD - 1),
                        )
                    alt_copy(kt_sb[:, 0, nt * QB : (nt + 1) * QB], ps)

            def projections(first=False):
                if first:
                    wide_wave()
                for w_sb, o_sb in ((wq_sb, qt_sb), (wk_sb, kt_sb)):
                    for c in range(1 if first else 0, 3):
                        for nt in range(NQ):
                            ps = avps.tile(
                                [P, QB], fp32,
                                tag="av" + "ABCD"[nt], name=f"pj{c}{nt}",
                            )
                            for k in range(KD):
                                nc.tensor.matmul(
                                    ps,
                                    lhsT=w_sb[:, k, c * P : (c + 1) * P],
                                    rhs=xT_sb[:, k, nt * QB : (nt + 1) * QB],
                                    start=(k == 0),
                                    stop=(k == KD - 1),
                                )
                            alt_copy(
                                o_sb[:, c, nt * QB : (nt + 1) * QB], ps
                            )

            def v_proj():
                # ---- V projection (natural layout) + ones columns ----
                # runs AFTER the first score block: its 15us of PE work
                # covers the ScalarE exp backlog for qj=3
                for m in range(MS):
                    ps = avps.tile(
                        [P, QB], fp32, tag="av" + "ABCD"[m % 4], name=f"pv{m}"
                    )
                    for k in range(KD):
                        nc.tensor.matmul(
                            ps[:, 0:HD],
                            lhsT=xT_sb[:, k, m * P : (m + 1) * P],
                            rhs=wv_sb[:, k, :],
                            start=(k == 0),
                            stop=(k == KD - 1),
                        )
                    (nc.scalar.copy if actcopy else nc.vector.tensor_copy)(
                        v_sb[:, m, 0:192], ps[:, 0:192]
                    )
                    nc.vector.tensor_copy(v_sb[:, m, 193:257], ps[:, 192:256])
                    nc.vector.tensor_copy(v_sb[:, m, 258:386], ps[:, 256:384])

            # ---- attention per q-block; out-proj deferred one block ----
            def out_proj(qj, mis=(0, 1, 2, 3), on_act=False, tail=False,
                         on_avps=False):
                Copy = mybir.ActivationFunctionType.Copy
                for mi in mis:
                    m = qj * 4 + mi
                    ost = ostp.tile([P, D], bf16, tag="ost")
                    rinv1 = rinv_sb[qj][:, mi : mi + 1]
                    for n in range(2):
                        # when the AV psum tags are idle, use them for the
                        # out-proj psum: the "sc" pool is recycled through
                        # the exp backlog and stalls the allocation
                        if on_avps:
                            ps = avps.tile(
                                [P, QB], fp32,
                                tag=("avB" if n == 0 else "avD"),
                                name=f"op{qj}{mi}{n}",
                            )
                        else:
                            ps = scps.tile(
                                [P, QB], fp32, tag="sc", name=f"op{mi}{n}"
                            )
                        for c in range(3):
                            nc.tensor.matmul(
                                ps[:, 0:384],
                                lhsT=pt_sb[:, c, m * P : (m + 1) * P],
                                rhs=wo_sb[:, c, n * 384 : (n + 1) * 384],
                                start=(c == 0),
                                stop=(c == 2),
                            )
                        # psum->sbuf move fused with the 1/(d0*d1) row scale
                        h = 2 * mi + n
                        if (h % 2 == 1) if tail else on_act:
                            nc.scalar.activation(
                                ost[:, n * 384 : (n + 1) * 384],
                                ps[:, 0:384], Copy, scale=rinv1,
                            )
                        else:
                            nc.vector.tensor_scalar_mul(
                                ost[:, n * 384 : (n + 1) * 384],
                                ps[:, 0:384], rinv1,
                            )
                        if tail:
                            # final block: drain each half immediately so
                            # the copy chain and DMAs overlap
                            nsl = slice(n * 384, (n + 1) * 384)
                            # sync/gpsimd only: a scalar-queue issue would
                            # serialize with the tail's ACT copies
                            [nc.sync, nc.gpsimd][h % 2].dma_start(
                                out[m * P : (m + 1) * P, nsl], ost[:, nsl]
                            )
                    if not tail:
                        # spread output DMAs over queues: the final drain
                        # barrier waits on them
                        # sync/gpsimd only: the ~0.7us issue slice on the
                        # scalar queue would delay the exp backlog
                        [nc.sync, nc.gpsimd][m % 2].dma_start(
                            out[m * P : (m + 1) * P, :], ost[:]
                        )

            def scores_part(qj):
                nk = 4 * qj + 4  # live key blocks (causal)
                # fused at tile: both heads side by side [.., h0 512 | h1 512]
                at2 = atp.tile(
                    [P, NK, 2 * QB], bf16, tag="at2", name=f"at2_{qj}"
                )
                for ki in range(nk):
                    ksl = slice(ki * KB, (ki + 1) * KB)
                    d = ki - 4 * qj
                    # diagonal blocks: queries below 128*d are fully masked
                    # -- trim the moving dim to the live suffix [t:QB]
                    t = max(0, d) * KB
                    qsl = slice(qj * QB + t, (qj + 1) * QB)
                    # one 2-bank psum tile holds both heads' score block
                    ps = scps.tile([P, 2 * QB], fp32, tag="sc")
                    ps0 = ps[:, t:QB]
                    ps1 = ps[:, QB + t : 2 * QB]
                    # full-plane matmuls (128 contraction rows); their
                    # LDWEIGHTS hide in the background weight buffer
                    nc.tensor.matmul(
                        ps0, lhsT=kt_sb[:, 0, ksl], rhs=qt_sb[:, 0, qsl],
                        start=True, stop=False,
                    )
                    nc.tensor.matmul(
                        ps1, lhsT=kt_sb[:, 1, ksl], rhs=qt_sb[:, 1, qsl],
                        start=True, stop=False,
                    )
                    # 64-row tails in disjoint row groups (concurrent)
                    nc.tensor.matmul(
                        ps0, lhsT=kt_sb[0:64, 2, ksl], rhs=qt_sb[0:64, 2, qsl],
                        start=False, stop=True,
                    )
                    nc.tensor.matmul(
                        ps1,
                        lhsT=kt_sb[64:128, 2, ksl],
                        rhs=qt_sb[64:128, 2, qsl],
                        start=False, stop=True,
                    )
                    # one exp for both heads: amortizes the ACT ramp.  For
                    # trimmed diagonal blocks a 2-segment AP covers just the
                    # live columns of both heads.
                    if t == 0:
                        nc.scalar.activation(
                            at2[:, ki, :], ps, Exp, scale=SCALE
                        )
                        a2m = at2[:, ki, :]
                        msk = masks[:, d, :] if d >= 0 else None
                    else:
                        psv = ps.rearrange("p (h q) -> p h q", h=2)[:, :, t:QB]
                        a2m = at2[:, ki, :].rearrange(
                            "p (h q) -> p h q", h=2
                        )[:, :, t:QB]
                        msk = masks[:, d, :].rearrange(
                            "p (h q) -> p h q", h=2
                        )[:, :, t:QB]
                        nc.scalar.activation(a2m, psv, Exp, scale=SCALE)
                    if d >= 0:
                        # multiplicative causal zeroing post-exp: bf16 SBUF
                        # DVE fast mode, off the PSUM->exp chain
                        nc.vector.tensor_mul(a2m, a2m, msk)
                return at2

            def av_part(qj, at2, mid=None):
                qsl = slice(qj * QB, (qj + 1) * QB)
                nk = 4 * qj + 4
                at0 = at2[:, :, 0:QB]
                at1 = at2[:, :, QB : 2 * QB]

                def av(h, at, c0sl, c1sl, tagA, tagC):
                    psc0 = avps.tile(
                        [P, QB], fp32, tag=tagA, name=f"av0_{qj}{h}"
                    )
                    psc1 = avps.tile(
                        [P, QB], fp32, tag=tagC, name=f"av1_{qj}{h}"
                    )
                    for ki in range(nk):
                        # diagonal blocks contribute zeros below q=128*d:
                        # skip those columns (they were masked out anyway)
                        t = max(0, ki - 4 * qj) * KB
                        nc.tensor.matmul(
                            psc0[:, t:QB], lhsT=v_sb[:, ki, c0sl],
                            rhs=at[:, ki, t:QB],
                            start=(ki == 0), stop=(ki == nk - 1),
                        )
                        nc.tensor.matmul(
                            psc1[0:65, t:QB], lhsT=v_sb[:, ki, c1sl],
                            rhs=at[:, ki, t:QB],
                            start=(ki == 0), stop=(ki == nk - 1),
                        )
                    return psc0, psc1

                # Division-free per-head normalization: scale head h's
                # preout by the OTHER head's raw denominator, then divide
                # the head-summed out-proj rows by d0*d1 (a per-partition
                # scalar there).  The only reciprocal runs on a PE-
                # transposed [128,4] tile -- 4 elems/lane, ~160ns.
                def norm(h, psc0, psc1, rcb_ps, d_other):
                    rcb = rcp.tile([P, QB], fp32, tag="rcb")
                    nc.tensor.matmul(
                        rcb_ps, lhsT=ones1[:], rhs=d_other[:],
                        start=True, stop=True,
                    )
                    # DVE, not ACT: ACT is still draining exps here
                    nc.vector.tensor_copy(rcb[:], rcb_ps)
                    nc.vector.tensor_mul(pt_sb[:, h, qsl], psc0, rcb[:])
                    # upper 64 head dims land in plane 2: h0 -> partitions
                    # 0:64, h1 -> partitions 64:128 (partition-shifted write)
                    if h == 0:
                        nc.vector.tensor_mul(
                            pt_sb[0:64, 2, qsl], psc1[0:64], rcb[0:64]
                        )
                    else:
                        nc.vector.tensor_mul(
                            pt_sb[64:128, 2, qsl], psc1[0:64], rcb[64:128]
                        )

                psA, psC = av(0, at0, V_H0C0, V_H0C1, "avA", "avC")
                psB, psD = av(1, at1, V_H1C0, V_H1C1, "avB", "avD")
                d0r = rcp.tile([1, QB], bf16, tag="d0r")
                d1r = rcp.tile([1, QB], bf16, tag="d1r")
                nc.vector.tensor_copy(d0r[:], psC[64:65, :])
                nc.vector.tensor_copy(d1r[:], psD[64:65, :])
                if mid is not None:
                    mid()  # independent PE work to cover the norm chain
                # both heads' broadcasts share ONE 2-bank slot so the other
                # score slot is free for the next block's first matmuls
                rcb2 = scps.tile(
                    [P, 2 * QB], fp32, tag="sc", name=f"rcb2_{qj}"
                )
                # head 1 first: its psums (avB/avD) are the ones the next
                # out_proj block reuses, so release them earliest
                norm(1, psB, psD, rcb2[:, QB : 2 * QB], d0r)
                norm(0, psA, psC, rcb2[:, 0:QB], d1r)
                # 1/(d0*d1) in transposed [q-part, chunk] layout for the
                # out-proj per-partition scale; bf16 lhsT keeps the
                # contraction-1 transpose matmuls at full rate
                d01 = rcp.tile([1, QB], bf16, tag="d01")
                nc.vector.tensor_mul(d01[:], d0r[:], d1r[:])
                dT = avps.tile([P, QB], fp32, tag="avC", name=f"dT_{qj}")
                for c in range(4):
                    nc.tensor.matmul(
                        dT[:, c : c + 1],
                        lhsT=d01[0:1, c * P : (c + 1) * P],
                        rhs=ones1[0:1, 0:1],
                        start=True, stop=True,
                    )
                nc.vector.reciprocal(rinv_sb[qj][:], dT[:, 0:4])

            # big q-blocks first; every AV waits one-block-deferred so the
            # next block's scores cover its exp tail, and out-proj halves
            # cover the norm chains
            for _rep in range(reps):
                projections(first=(_rep == 0))
                a3 = scores_part(3)
                v_proj()
                a2 = scores_part(2)
                av_part(3, a3)
                a1 = scores_part(1)
                out_proj(3, (0, 1), on_avps=True)
                av_part(2, a2)
                a0 = scores_part(0)
                out_proj(3, (2, 3), on_avps=True)
                out_proj(2, (0, 1), on_avps=True)
                av_part(1, a1)
                out_proj(2, (2, 3))
                av_part(
                    0, a0,
                    mid=lambda: out_proj(1, (0, 1), on_act=True),
                )
                out_proj(1, (2, 3), on_act=True)
                out_proj(0, tail=True, on_avps=True)

    nc.compile()
    return nc


def _shard_inputs(x, Wq, Wk, Wv, Wo):
    bf = ml_dtypes.bfloat16
    in_maps = []
    for core in range(8):
        b, hp = core // 2, core % 2
        cols = slice(hp * HD, (hp + 1) * HD)
        in_maps.append(
            {
                "xT": np.ascontiguousarray(x[b].T).astype(bf),
                "wqT": np.ascontiguousarray(Wq[cols, :].T[:, PQ]).astype(bf),
                "wkT": np.ascontiguousarray(Wk[cols, :].T[:, PQ]).astype(bf),
                "wvT": np.ascontiguousarray(Wv[cols, :].T[:, PV]).astype(bf),
                "woS": np.ascontiguousarray(Wo[:, cols].T[PQ, :]).astype(bf),
            }
        )
    return in_maps


def _run(inputs, trace=False, **kw):
    from concourse.bass_utils import run_bass_kernel_spmd

    if "nc" not in _CACHED:
        _CACHED["nc"] = build_nc()
    nc = _CACHED["nc"]
    in_maps = _shard_inputs(
        np.asarray(inputs["x"], np.float32),
        np.asarray(inputs["Wq"], np.float32),
        np.asarray(inputs["Wk"], np.float32),
        np.asarray(inputs["Wv"], np.float32),
        np.asarray(inputs["Wo"], np.float32),
    )
    res = run_bass_kernel_spmd(
        nc, in_maps, core_ids=list(range(8)), trace=trace, **kw
    )
    parts = [np.asarray(r["out"], np.float32) for r in res.results]
    full = np.empty((B, S, D), np.float32)
    for b in range(B):
        full[b] = parts[2 * b] + parts[2 * b + 1]
    return full, res


def kernel(**inputs) -> np.ndarray:
    full, _ = _run(inputs, trace=False)
    return full

